# revision 1
# baseline (speedup 1.0000x reference)
"""Hamiltonian block-generation layer on 8 Trainium2 NeuronCores.

Strategy: shard the pair dimension P=130816 across 8 cores (16352 pairs each,
padded to 16384). The host pre-gathers transposed MLP inputs
xT = concat(n_i, n_j, e_ij)^T  [384, 16384] per core, so the device kernel is
identical on every core (pure SPMD):

  stage 1:  hoT[h, p] = silu(Wo1^T @ xT + bo1)        (2x matmul-accum chains)
  stage 2:  out_u = hoT^T @ Wo2  + (overlap + bo2)     (i,j block, row-major)
            out_t = hoT^T @ Wo2p + (overlap^T + bo2p)  (j,i block = transposed
                                                        block, via host-permuted
                                                        second-layer weights)
  diag:     batched separately (64 atoms/core) with W1/W2.

Outputs stay in pair-block layout [16384, 196] (perfectly coalesced DMA); the
host scatters blocks into the dense H [7168, 7168].
"""

import numpy as np
import ml_dtypes

BF16 = ml_dtypes.bfloat16

N_ATOMS = 512
B = 14
BB = B * B          # 196
F = 128
FE = 128
HID = 256
P = N_ATOMS * (N_ATOMS - 1) // 2   # 130816
NCORES = 8
PPC = P // NCORES                  # 16352 pairs per core
NB = 512                           # pairs per batch
NBATCH = (PPC + NB - 1) // NB      # 32
PPCP = NBATCH * NB                 # 16384 padded
DPC = N_ATOMS // NCORES            # 64 diag atoms per core

_CACHE = {}


def _build_nc():
    import concourse.mybir as mybir
    import concourse.tile as tile
    from concourse import bacc

    f32 = mybir.dt.float32
    nc = bacc.Bacc("TRN2", target_bir_lowering=False)

    bf16 = mybir.dt.bfloat16
    xT = nc.dram_tensor("xT", [3 * F, PPCP], bf16, kind="ExternalInput")
    ovu = nc.dram_tensor("ovu", [PPCP, BB], f32, kind="ExternalInput")
    xdT = nc.dram_tensor("xdT", [HID, DPC], bf16, kind="ExternalInput")
    apd = nc.dram_tensor("apd", [DPC, BB], f32, kind="ExternalInput")
    Wo1 = nc.dram_tensor("Wo1", [3 * F, HID], bf16, kind="ExternalInput")
    W1 = nc.dram_tensor("W1", [HID, HID], bf16, kind="ExternalInput")
    Wo2c = nc.dram_tensor("Wo2c", [HID, 2 * BB], bf16, kind="ExternalInput")
    W2 = nc.dram_tensor("W2", [HID, BB], bf16, kind="ExternalInput")
    b1 = nc.dram_tensor("b1", [2, 128], f32, kind="ExternalInput")
    bo1 = nc.dram_tensor("bo1", [2, 128], f32, kind="ExternalInput")

    out_u = nc.dram_tensor("out_u", [PPCP, BB], f32, kind="ExternalOutput")
    out_t = nc.dram_tensor("out_t", [PPCP, BB], f32, kind="ExternalOutput")
    out_d = nc.dram_tensor("out_d", [DPC, BB], f32, kind="ExternalOutput")

    import concourse.bass as bass  # noqa: F401

    silu = mybir.ActivationFunctionType.Silu

    with tile.TileContext(nc) as tc:
        with tc.tile_pool(name="consts", bufs=1) as consts, \
             tc.tile_pool(name="xin", bufs=3) as xin, \
             tc.tile_pool(name="hpool", bufs=2) as hpool, \
             tc.tile_pool(name="ovp", bufs=3) as ovp, \
             tc.tile_pool(name="outp", bufs=3) as outp, \
             tc.tile_pool(name="psH", bufs=2, space="PSUM") as psH, \
             tc.tile_pool(name="psO", bufs=4, space="PSUM") as psO:

            # ---- weights into SBUF, K-chunked: [128, nchunk, out_dim]
            wo1 = consts.tile([128, 3, HID], bf16, tag="wo1")
            nc.sync.dma_start(out=wo1, in_=Wo1.rearrange("(c p) h -> p c h", p=128))
            w1 = consts.tile([128, 2, HID], bf16, tag="w1")
            nc.sync.dma_start(out=w1, in_=W1.rearrange("(c p) h -> p c h", p=128))
            wo2c = consts.tile([128, 2, 2 * BB], bf16, tag="wo2c")
            nc.sync.dma_start(out=wo2c, in_=Wo2c.rearrange("(c p) e -> p c e", p=128))
            w2 = consts.tile([128, 2, BB], bf16, tag="w2")
            nc.sync.dma_start(out=w2, in_=W2.rearrange("(c p) e -> p c e", p=128))
            b1t = consts.tile([128, 2], f32, tag="b1t")
            nc.sync.dma_start(out=b1t, in_=b1.rearrange("c p -> p c"))
            bo1t = consts.tile([128, 2], f32, tag="bo1t")
            nc.sync.dma_start(out=bo1t, in_=bo1.rearrange("c p -> p c"))

            # ---- main pair loop
            for bi in range(NBATCH):
                base = bi * NB
                rhs = []
                for k in range(3):
                    r = xin.tile([128, NB], bf16, tag=f"rhs{k}")
                    nc.sync.dma_start(
                        out=r, in_=xT[k * 128:(k + 1) * 128, base:base + NB])
                    rhs.append(r)
                ho = []
                for h in range(2):
                    ph = psH.tile([128, NB], f32, tag=f"psh{h}")
                    for k in range(3):
                        nc.tensor.matmul(
                            ph, wo1[:, k, h * 128:(h + 1) * 128], rhs[k],
                            start=(k == 0), stop=(k == 2))
                    hs = hpool.tile([128, NB], bf16, tag=f"ho{h}")
                    nc.scalar.activation(hs, ph, silu, bias=bo1t[:, h:h + 1])
                    ho.append(hs)
                for pt in range(NB // 128):
                    row = base + pt * 128
                    ov = ovp.tile([128, BB], f32, tag="ov")
                    nc.sync.dma_start(out=ov, in_=ovu[row:row + 128, :])
                    ps = psO.tile([128, 2 * BB], f32, tag="pso")
                    for h in range(2):
                        nc.tensor.matmul(
                            ps, ho[h][:, pt * 128:(pt + 1) * 128],
                            wo2c[:, h, :], start=(h == 0), stop=(h == 1))
                    for (off, o_dram, swapped) in (
                            (0, out_u, False), (BB, out_t, True)):
                        if swapped:
                            ov_in = ov[:, :].rearrange("p (v u) -> p u v", v=B)
                        else:
                            ov_in = ov[:, :].rearrange("p (u v) -> p u v", u=B)
                        ot = outp.tile([128, BB], f32, tag="ot")
                        nc.vector.tensor_add(
                            ot[:, :].rearrange("p (u v) -> p u v", u=B),
                            ps[:, off:off + BB].rearrange("p (u v) -> p u v", u=B),
                            ov_in)
                        nc.sync.dma_start(out=o_dram[row:row + 128, :], in_=ot)

            # ---- diagonal blocks (64 atoms)
            rd = xin.tile([128, 2, DPC], bf16, tag="rhsd")
            nc.sync.dma_start(out=rd, in_=xdT.rearrange("(c p) a -> p c a", p=128))
            hod = []
            for h in range(2):
                ph = psH.tile([128, DPC], f32, tag=f"psh{h}")
                for k in range(2):
                    nc.tensor.matmul(
                        ph, w1[:, k, h * 128:(h + 1) * 128], rd[:, k, :],
                        start=(k == 0), stop=(k == 1))
                hs = hpool.tile([128, DPC], bf16, tag=f"hod{h}")
                nc.scalar.activation(hs, ph, silu, bias=b1t[:, h:h + 1])
                hod.append(hs)
            psd = psO.tile([DPC, BB], f32, tag="pso")
            for h in range(2):
                nc.tensor.matmul(psd, hod[h], w2[:, h, :],
                                 start=(h == 0), stop=(h == 1))
            apt = ovp.tile([DPC, BB], f32, tag="ov")
            nc.sync.dma_start(out=apt, in_=apd[:, :])
            otd = outp.tile([DPC, BB], f32, tag="ot")
            nc.vector.tensor_add(otd, psd, apt)
            nc.sync.dma_start(out=out_d[:, :], in_=otd)

    nc.finalize()
    return nc


def kernel(**inputs) -> np.ndarray:
    nodes_feature = np.ascontiguousarray(np.asarray(inputs["nodes_feature"], np.float32))
    edges_feature = np.asarray(inputs["edges_feature"], np.float32)
    atom_blocks = np.asarray(inputs["atom_blocks"], np.float32)
    overlap_pair = np.asarray(inputs["overlap_pair"], np.float32)
    W1 = np.ascontiguousarray(np.asarray(inputs["W1"], np.float32))
    b1 = np.asarray(inputs["b1"], np.float32)
    W2 = np.ascontiguousarray(np.asarray(inputs["W2"], np.float32))
    b2 = np.asarray(inputs["b2"], np.float32)
    Wo1 = np.ascontiguousarray(np.asarray(inputs["Wo1"], np.float32))
    bo1 = np.asarray(inputs["bo1"], np.float32)
    Wo2 = np.ascontiguousarray(np.asarray(inputs["Wo2"], np.float32))
    bo2 = np.asarray(inputs["bo2"], np.float32)
    pair_i = np.asarray(inputs["pair_i"]).astype(np.int64)
    pair_j = np.asarray(inputs["pair_j"]).astype(np.int64)

    # ---- host prep
    nodesT = np.ascontiguousarray(nodes_feature.T)                 # [128, 512]
    e = np.arange(BB)
    perm = (e % B) * B + e // B                                    # transpose perm
    Wo2c = np.ascontiguousarray(
        np.concatenate([Wo2, Wo2[:, perm]], axis=1)).astype(BF16)
    bo2p = bo2[perm]
    ar = np.arange(N_ATOMS)
    eaa = edges_feature[ar, ar]                                    # [512, 128]

    in_maps = []
    for m in range(NCORES):
        sel = slice(m * PPC, (m + 1) * PPC)
        pi, pj = pair_i[sel], pair_j[sel]
        xT = np.zeros((3 * F, PPCP), BF16)
        xT[0:128, :PPC] = nodesT[:, pi].astype(BF16)
        xT[128:256, :PPC] = nodesT[:, pj].astype(BF16)
        xT[256:384, :PPC] = edges_feature[pi, pj].T.astype(BF16)
        ovu = np.zeros((PPCP, BB), np.float32)
        ovu[:PPC] = overlap_pair[sel].reshape(-1, BB) + bo2
        d = slice(m * DPC, (m + 1) * DPC)
        xdT = np.empty((HID, DPC), np.float32)
        xdT[0:128] = nodesT[:, d]
        xdT[128:256] = eaa[d].T
        apd = atom_blocks[d].reshape(-1, BB) + b2
        in_maps.append({
            "xT": xT, "ovu": ovu,
            "xdT": np.ascontiguousarray(xdT).astype(BF16),
            "apd": np.ascontiguousarray(apd),
            "Wo1": Wo1.astype(BF16), "W1": W1.astype(BF16),
            "Wo2c": Wo2c, "W2": W2.astype(BF16),
            "b1": np.ascontiguousarray(b1.reshape(2, 128)),
            "bo1": np.ascontiguousarray(bo1.reshape(2, 128)),
        })

    if "nc" not in _CACHE:
        _CACHE["nc"] = _build_nc()
    nc = _CACHE["nc"]

    import os
    import time
    from concourse.bass_utils import run_bass_kernel_spmd
    trace = bool(int(os.environ.get("KERNEL_TRACE", "0")))
    t0 = time.time()
    if trace:
        try:
            res = run_bass_kernel_spmd(nc, in_maps, core_ids=list(range(NCORES)),
                                       trace=True)
        except Exception:
            res = run_bass_kernel_spmd(nc, in_maps, core_ids=list(range(NCORES)))
    else:
        res = run_bass_kernel_spmd(nc, in_maps, core_ids=list(range(NCORES)))
    _CACHE["run_wall_s"] = time.time() - t0
    _CACHE["last_result"] = res

    # ---- host scatter into dense H
    H4 = np.zeros((N_ATOMS, B, N_ATOMS, B), np.float32)
    all_u = np.concatenate([res.results[m]["out_u"][:PPC] for m in range(NCORES)])
    all_t = np.concatenate([res.results[m]["out_t"][:PPC] for m in range(NCORES)])
    all_d = np.concatenate([res.results[m]["out_d"] for m in range(NCORES)])
    H4[pair_i, :, pair_j, :] = all_u.reshape(-1, B, B)
    H4[pair_j, :, pair_i, :] = all_t.reshape(-1, B, B)
    ar = np.arange(N_ATOMS)
    H4[ar, :, ar, :] = all_d.reshape(-1, B, B)
    return H4.reshape(N_ATOMS * B, N_ATOMS * B)



# revision 2
# speedup vs baseline: 7.9287x; 7.9287x over previous
"""Hamiltonian block-generation layer on 8 Trainium2 NeuronCores.

Strategy (v2, transfer-minimal): shard the pair dimension P=130816 across 8
cores (16352 pairs each, padded to 16384). The device computes ONLY the
off-diagonal MLP delta mo = silu(x @ Wo1 + bo1) @ Wo2 per pair, in fp8-e4m3
input / fp8-e4m3 output precision (quantization error lands ~1e-3 relative to
the Hamiltonian's absmax, well under the 2e-2 gate). Everything that is cheap
on host and expensive to ship stays on host:

  - overlap_pair (100 MB) is never uploaded; host adds overlap + bo2 + mo.
  - the transposed (j,i) block is never shipped; host writes mo^T itself.
  - the diagonal-block MLP (512 atoms, 59 MFLOP) runs in numpy, exact f32.

Per-core device I/O: xT [384, 16384] fp8 (6.3 MB) in, mo [16384, 196] fp8
(3.2 MB) out — ~77 MB total across 8 cores vs ~410 MB for the naive scheme.
The axon tunnel (~50-95 MB/s) is the bottleneck, so bytes == seconds.
"""

import os

import numpy as np
import ml_dtypes

F8 = ml_dtypes.float8_e4m3
BF16 = ml_dtypes.bfloat16

N_ATOMS = 512
B = 14
BB = B * B          # 196
F = 128
FE = 128
HID = 256
P = N_ATOMS * (N_ATOMS - 1) // 2   # 130816
NCORES = 8
PPC = P // NCORES                  # 16352 pairs per core
NB = 512                           # pairs per batch
NBATCH = (PPC + NB - 1) // NB      # 32
PPCP = NBATCH * NB                 # 16384 padded

IN_DT = os.environ.get("KERNEL_IN_DT", "float8e4")
OUT_DT = os.environ.get("KERNEL_OUT_DT", "float8e4")

_CACHE = {}


def _np_dt(name):
    return {"float8e4": F8, "bfloat16": BF16, "float32": np.float32}[name]


def _build_nc(in_dt_name, out_dt_name):
    import concourse.mybir as mybir
    import concourse.tile as tile
    from concourse import bacc

    f32 = mybir.dt.float32
    bf16 = mybir.dt.bfloat16
    in_dt = getattr(mybir.dt, in_dt_name)
    out_dt = getattr(mybir.dt, out_dt_name)

    nc = bacc.Bacc("TRN2", target_bir_lowering=False)

    xT = nc.dram_tensor("xT", [3 * F, PPCP], in_dt, kind="ExternalInput")
    Wo1 = nc.dram_tensor("Wo1", [3 * F, HID], in_dt, kind="ExternalInput")
    Wo2 = nc.dram_tensor("Wo2", [HID, BB], bf16, kind="ExternalInput")
    bo1 = nc.dram_tensor("bo1", [2, 128], f32, kind="ExternalInput")
    mo = nc.dram_tensor("mo", [PPCP, BB], out_dt, kind="ExternalOutput")

    silu = mybir.ActivationFunctionType.Silu

    with tile.TileContext(nc) as tc:
        with tc.tile_pool(name="consts", bufs=1) as consts, \
             tc.tile_pool(name="xin", bufs=3) as xin, \
             tc.tile_pool(name="hpool", bufs=2) as hpool, \
             tc.tile_pool(name="outp", bufs=3) as outp, \
             tc.tile_pool(name="psH", bufs=2, space="PSUM") as psH, \
             tc.tile_pool(name="psO", bufs=4, space="PSUM") as psO:

            # ---- weights into SBUF, K-chunked: [128, nchunk, out_dim]
            wo1 = consts.tile([128, 3, HID], in_dt, tag="wo1")
            nc.sync.dma_start(out=wo1, in_=Wo1.rearrange("(c p) h -> p c h", p=128))
            wo2 = consts.tile([128, 2, BB], bf16, tag="wo2")
            nc.sync.dma_start(out=wo2, in_=Wo2.rearrange("(c p) e -> p c e", p=128))
            bo1t = consts.tile([128, 2], f32, tag="bo1t")
            nc.sync.dma_start(out=bo1t, in_=bo1.rearrange("c p -> p c"))

            # ---- main pair loop
            for bi in range(NBATCH):
                base = bi * NB
                rhs = []
                for k in range(3):
                    r = xin.tile([128, NB], in_dt, tag=f"rhs{k}")
                    nc.sync.dma_start(
                        out=r, in_=xT[k * 128:(k + 1) * 128, base:base + NB])
                    rhs.append(r)
                ho = []
                for h in range(2):
                    ph = psH.tile([128, NB], f32, tag=f"psh{h}")
                    for k in range(3):
                        nc.tensor.matmul(
                            ph, wo1[:, k, h * 128:(h + 1) * 128], rhs[k],
                            start=(k == 0), stop=(k == 2))
                    hs = hpool.tile([128, NB], bf16, tag=f"ho{h}")
                    nc.scalar.activation(hs, ph, silu, bias=bo1t[:, h:h + 1])
                    ho.append(hs)
                for pt in range(NB // 128):
                    row = base + pt * 128
                    ps = psO.tile([128, BB], f32, tag="pso")
                    for h in range(2):
                        nc.tensor.matmul(
                            ps, ho[h][:, pt * 128:(pt + 1) * 128],
                            wo2[:, h, :], start=(h == 0), stop=(h == 1))
                    ot = outp.tile([128, BB], out_dt, tag="ot")
                    nc.scalar.copy(ot, ps)
                    nc.sync.dma_start(out=mo[row:row + 128, :], in_=ot)

    nc.finalize()
    return nc


def kernel(**inputs) -> np.ndarray:
    nodes = np.ascontiguousarray(np.asarray(inputs["nodes_feature"], np.float32))
    edges = np.asarray(inputs["edges_feature"], np.float32)
    atom_blocks = np.asarray(inputs["atom_blocks"], np.float32)
    overlap = np.asarray(inputs["overlap_pair"], np.float32)
    W1 = np.asarray(inputs["W1"], np.float32)
    b1 = np.asarray(inputs["b1"], np.float32)
    W2 = np.asarray(inputs["W2"], np.float32)
    b2 = np.asarray(inputs["b2"], np.float32)
    Wo1 = np.ascontiguousarray(np.asarray(inputs["Wo1"], np.float32))
    bo1 = np.asarray(inputs["bo1"], np.float32)
    Wo2 = np.ascontiguousarray(np.asarray(inputs["Wo2"], np.float32))
    bo2 = np.asarray(inputs["bo2"], np.float32)
    pair_i = np.asarray(inputs["pair_i"]).astype(np.int64)
    pair_j = np.asarray(inputs["pair_j"]).astype(np.int64)

    in_np = _np_dt(IN_DT)

    # ---- host prep: per-pair MLP inputs, transposed, quantized
    nodesT_q = np.ascontiguousarray(nodes.T).astype(in_np)         # [128, 512]
    eg_q = edges[pair_i, pair_j].astype(in_np)                     # [P, 128]
    Wo1_q = Wo1.astype(in_np)
    Wo2_bf = Wo2.astype(BF16)
    bo1_2 = np.ascontiguousarray(bo1.reshape(2, 128))

    in_maps = []
    for m in range(NCORES):
        sel = slice(m * PPC, (m + 1) * PPC)
        xT = np.zeros((3 * F, PPCP), in_np)
        xT[0:128, :PPC] = nodesT_q[:, pair_i[sel]]
        xT[128:256, :PPC] = nodesT_q[:, pair_j[sel]]
        xT[256:384, :PPC] = eg_q[sel].T
        in_maps.append({"xT": xT, "Wo1": Wo1_q, "Wo2": Wo2_bf, "bo1": bo1_2})

    key = ("nc", IN_DT, OUT_DT)
    if key not in _CACHE:
        _CACHE[key] = _build_nc(IN_DT, OUT_DT)
    nc = _CACHE[key]

    import time
    from concourse.bass_utils import run_bass_kernel_spmd
    trace = bool(int(os.environ.get("KERNEL_TRACE", "0")))
    t0 = time.time()
    if trace:
        try:
            res = run_bass_kernel_spmd(nc, in_maps, core_ids=list(range(NCORES)),
                                       trace=True)
        except Exception:
            res = run_bass_kernel_spmd(nc, in_maps, core_ids=list(range(NCORES)))
    else:
        res = run_bass_kernel_spmd(nc, in_maps, core_ids=list(range(NCORES)))
    _CACHE["run_wall_s"] = time.time() - t0
    _CACHE["last_result"] = res

    # ---- host epilogue: add overlap + bias, diag MLP, scatter into dense H
    mo = np.concatenate(
        [res.results[m]["mo"][:PPC] for m in range(NCORES)]).astype(np.float32)
    off = overlap.reshape(P, BB) + mo + bo2                        # [P, 196]

    ar = np.arange(N_ATOMS)
    xd = np.concatenate([nodes, edges[ar, ar]], axis=1)            # [512, 256]
    zd = xd @ W1 + b1
    hd = zd / (1.0 + np.exp(-zd))                                  # silu, f32
    md = hd @ W2 + b2
    dblk = atom_blocks + md.reshape(-1, B, B)

    H4 = np.zeros((N_ATOMS, B, N_ATOMS, B), np.float32)
    H4[ar, :, ar, :] = dblk
    offb = off.reshape(P, B, B)
    H4[pair_i, :, pair_j, :] = offb
    H4[pair_j, :, pair_i, :] = offb.transpose(0, 2, 1)
    return H4.reshape(N_ATOMS * B, N_ATOMS * B)


# revision 3
# speedup vs baseline: 9.0085x; 1.1362x over previous
"""Hamiltonian block-generation layer on 8 Trainium2 NeuronCores.

Strategy (v3, transfer-minimal): the axon tunnel (~65-145 MB/s) dominates, so
the kernel ships the minimum bytes that carry real information.

Pair sharding exploits the triu structure: pairs (i, j>i) with the same i form
a contiguous run. Core m takes rows i == m (mod 8); slot k on every core has
the same padded width W_k = 16*ceil((511-8k)/16) (sum 16896 = 132*128), so the
program is identical across cores (pure SPMD) while the data differs.

Per slot (row i), the stage-1 pre-activation  x @ Wo1  splits into
  -  e_ij  @ Wo1_e : per-pair matmul over the uploaded e^T tile,
  -  n_j   @ Wo1_n : matmul against a contiguous slice of a small per-core
                     shifted node table (n_j runs j = i+1 .. 511),
  -  n_i   @ Wo1_i + bo1 : constant per slot -> host-computed f32 bias vector
                     fed to the Silu activation (64 KB upload).
so the 33 MB of replicated per-pair node features is never transferred.

The device returns only the off-diagonal MLP delta mo (fp8-e4m3, 3.3 MB/core);
host adds overlap + bo2, writes both block orientations, and runs the tiny
diagonal MLP in exact f32 numpy. fp8 quantization lands ~4e-3 relative to the
Hamiltonian absmax, well under the 2e-2 gate.

If pair_i/pair_j are not the standard lexicographic triu enumeration the
kernel falls back to an exact host-side computation (never triggers for the
reference's setup_inputs).
"""

import os

import numpy as np
import ml_dtypes

F8 = ml_dtypes.float8_e4m3
BF16 = ml_dtypes.bfloat16

N_ATOMS = 512
B = 14
BB = B * B          # 196
F = 128
FE = 128
HID = 256
P = N_ATOMS * (N_ATOMS - 1) // 2   # 130816
NCORES = 8

# slot layout: core m, slot k -> row i = m + 8k, true width 511 - i,
# padded width W[k] = 16*ceil((511-8k)/16)  (same on every core)
_KS = np.arange(64)
W_SLOT = (16 * np.ceil((511 - 8 * _KS) / 16)).astype(np.int64)
OFF_SLOT = np.concatenate([[0], np.cumsum(W_SLOT)])[:-1]
COLS = int(W_SLOT.sum())           # 16896 = 132*128
assert COLS % 128 == 0
NJP = 528                          # shifted node table width (>= max 8k+W_k)

IN_DT = os.environ.get("KERNEL_IN_DT", "float8e4")
OUT_DT = os.environ.get("KERNEL_OUT_DT", "float8e4")

_CACHE = {}


def _np_dt(name):
    return {"float8e4": F8, "bfloat16": BF16, "float32": np.float32}[name]


def _build_nc(in_dt_name, out_dt_name):
    import concourse.mybir as mybir
    import concourse.tile as tile
    from concourse import bacc

    f32 = mybir.dt.float32
    bf16 = mybir.dt.bfloat16
    in_dt = getattr(mybir.dt, in_dt_name)
    out_dt = getattr(mybir.dt, out_dt_name)

    nc = bacc.Bacc("TRN2", target_bir_lowering=False)

    eT = nc.dram_tensor("eT", [128, COLS], in_dt, kind="ExternalInput")
    nodesJ = nc.dram_tensor("nodesJ", [128, NJP], in_dt, kind="ExternalInput")
    Ab = nc.dram_tensor("Ab", [128, 2 * 64], f32, kind="ExternalInput")
    Wo1 = nc.dram_tensor("Wo1", [2 * F, HID], in_dt, kind="ExternalInput")
    Wo2 = nc.dram_tensor("Wo2", [HID, BB], bf16, kind="ExternalInput")
    mo = nc.dram_tensor("mo", [COLS, BB], out_dt, kind="ExternalOutput")

    silu = mybir.ActivationFunctionType.Silu

    with tile.TileContext(nc) as tc:
        with tc.tile_pool(name="consts", bufs=1) as consts, \
             tc.tile_pool(name="outp", bufs=3) as outp, \
             tc.tile_pool(name="psH", bufs=2, space="PSUM") as psH, \
             tc.tile_pool(name="psO", bufs=4, space="PSUM") as psO:

            # ---- resident inputs
            et = consts.tile([128, COLS], in_dt, tag="et")
            nc.sync.dma_start(out=et, in_=eT[:, :])
            nj = consts.tile([128, NJP], in_dt, tag="nj")
            nc.sync.dma_start(out=nj, in_=nodesJ[:, :])
            abt = consts.tile([128, 2 * 64], f32, tag="abt")
            nc.sync.dma_start(out=abt, in_=Ab[:, :])
            wo1 = consts.tile([128, 2, HID], in_dt, tag="wo1")
            nc.sync.dma_start(out=wo1, in_=Wo1.rearrange("(c p) h -> p c h", p=128))
            wo2 = consts.tile([128, 2, BB], bf16, tag="wo2")
            nc.sync.dma_start(out=wo2, in_=Wo2.rearrange("(c p) e -> p c e", p=128))
            hob = consts.tile([128, 2, COLS], bf16, tag="hob")

            # ---- stage 1: ho^T = silu(Wo1_n^T nj + Wo1_e^T e + A_i + bo1)
            for k in range(64):
                off = int(OFF_SLOT[k])
                w = int(W_SLOT[k])
                for h in range(2):
                    ph = psH.tile([128, 512], f32, tag=f"psh{h}")
                    nc.tensor.matmul(
                        ph[:, :w], wo1[:, 1, h * 128:(h + 1) * 128],
                        et[:, off:off + w], start=True, stop=False)
                    nc.tensor.matmul(
                        ph[:, :w], wo1[:, 0, h * 128:(h + 1) * 128],
                        nj[:, 8 * k:8 * k + w], start=False, stop=True)
                    nc.scalar.activation(
                        hob[:, h, off:off + w], ph[:, :w], silu,
                        bias=abt[:, h * 64 + k:h * 64 + k + 1])

            # ---- stage 2: mo = ho^T.T @ Wo2
            for t in range(COLS // 128):
                ps = psO.tile([128, BB], f32, tag="pso")
                for h in range(2):
                    nc.tensor.matmul(
                        ps, hob[:, h, t * 128:(t + 1) * 128],
                        wo2[:, h, :], start=(h == 0), stop=(h == 1))
                ot = outp.tile([128, BB], out_dt, tag="ot")
                nc.scalar.copy(ot, ps)
                nc.sync.dma_start(out=mo[t * 128:(t + 1) * 128, :], in_=ot)

    nc.finalize()
    return nc


def _triu_maps():
    """Device-order <-> input-order index maps (input = lexicographic triu).

    Returns (dev_idx, inp_idx): mo_global[dev_idx] are the valid device rows,
    belonging to triu positions inp_idx.
    """
    if "maps" in _CACHE:
        return _CACHE["maps"]
    base = np.concatenate([[0], np.cumsum(511 - np.arange(512))])[:-1]  # [512]
    dev_idx = []
    inp_idx = []
    for m in range(NCORES):
        rows = m + 8 * _KS                       # [64]
        for k in range(64):
            i = int(rows[k])
            L = 511 - i
            if L <= 0:
                continue
            dev_idx.append(m * COLS + int(OFF_SLOT[k]) + np.arange(L))
            inp_idx.append(int(base[i]) + np.arange(L))
    maps = (np.concatenate(dev_idx), np.concatenate(inp_idx))
    _CACHE["maps"] = maps
    return maps


def _silu(z):
    return z / (1.0 + np.exp(-z))


def _host_fallback(nodes, edges, overlap, Wo1, bo1, Wo2, bo2, pair_i, pair_j):
    """Exact f32 off-diagonal blocks for arbitrary pair lists."""
    out = np.empty((len(pair_i), BB), np.float32)
    CH = 8192
    for s in range(0, len(pair_i), CH):
        pi = pair_i[s:s + CH]
        pj = pair_j[s:s + CH]
        x = np.concatenate(
            [nodes[pi], nodes[pj], edges[pi, pj]], axis=1)
        out[s:s + CH] = _silu(x @ Wo1 + bo1) @ Wo2 + bo2
    return overlap.reshape(-1, BB) + out


def kernel(**inputs) -> np.ndarray:
    nodes = np.ascontiguousarray(np.asarray(inputs["nodes_feature"], np.float32))
    edges = np.asarray(inputs["edges_feature"], np.float32)
    atom_blocks = np.asarray(inputs["atom_blocks"], np.float32)
    overlap = np.asarray(inputs["overlap_pair"], np.float32)
    W1 = np.asarray(inputs["W1"], np.float32)
    b1 = np.asarray(inputs["b1"], np.float32)
    W2 = np.asarray(inputs["W2"], np.float32)
    b2 = np.asarray(inputs["b2"], np.float32)
    Wo1 = np.ascontiguousarray(np.asarray(inputs["Wo1"], np.float32))
    bo1 = np.asarray(inputs["bo1"], np.float32)
    Wo2 = np.ascontiguousarray(np.asarray(inputs["Wo2"], np.float32))
    bo2 = np.asarray(inputs["bo2"], np.float32)
    pair_i = np.asarray(inputs["pair_i"]).astype(np.int64)
    pair_j = np.asarray(inputs["pair_j"]).astype(np.int64)

    tri, trj = np.triu_indices(N_ATOMS, k=1)
    is_triu = np.array_equal(pair_i, tri) and np.array_equal(pair_j, trj)

    if is_triu:
        off = _device_off_blocks(nodes, edges, overlap, Wo1, bo1, Wo2, bo2)
    else:
        off = _host_fallback(nodes, edges, overlap, Wo1, bo1, Wo2, bo2,
                             pair_i, pair_j)

    # ---- diagonal blocks (exact f32, host)
    ar = np.arange(N_ATOMS)
    xd = np.concatenate([nodes, edges[ar, ar]], axis=1)            # [512, 256]
    md = _silu(xd @ W1 + b1) @ W2 + b2
    dblk = atom_blocks + md.reshape(-1, B, B)

    # ---- scatter into dense H
    H4 = np.zeros((N_ATOMS, B, N_ATOMS, B), np.float32)
    H4[ar, :, ar, :] = dblk
    offb = off.reshape(P, B, B)
    H4[pair_i, :, pair_j, :] = offb
    H4[pair_j, :, pair_i, :] = offb.transpose(0, 2, 1)
    return H4.reshape(N_ATOMS * B, N_ATOMS * B)


def _device_off_blocks(nodes, edges, overlap, Wo1, bo1, Wo2, bo2):
    in_np = _np_dt(IN_DT)

    # ---- host prep
    nodesT_q = np.ascontiguousarray(nodes.T).astype(in_np)         # [128, 512]
    e_q = edges.astype(in_np)                                      # [512,512,128]
    Wo1_q = np.ascontiguousarray(Wo1[128:384]).astype(in_np)       # [256, 256]
    Wo2_bf = Wo2.astype(BF16)
    A_all = nodes @ Wo1[:128] + bo1                                # [512, 256] f32

    in_maps = []
    ebuf = np.zeros((COLS, 128), in_np)
    for m in range(NCORES):
        rows = m + 8 * _KS
        ebuf[:] = 0
        for k in range(64):
            i = int(rows[k])
            L = 511 - i
            if L > 0:
                o = int(OFF_SLOT[k])
                ebuf[o:o + L] = e_q[i, i + 1:512]
        eT = np.ascontiguousarray(ebuf.T)                          # [128, COLS]
        nodesJ = np.zeros((128, NJP), in_np)
        nodesJ[:, :511 - m] = nodesT_q[:, m + 1:]
        # bias table: Ab[p, h*64+k] = A_all[row_k, h*128+p]
        Ab = np.ascontiguousarray(
            A_all[rows].reshape(64, 2, 128).transpose(2, 1, 0).reshape(128, 128))
        in_maps.append({"eT": eT, "nodesJ": nodesJ, "Ab": Ab,
                        "Wo1": Wo1_q, "Wo2": Wo2_bf})

    key = ("nc", IN_DT, OUT_DT)
    if key not in _CACHE:
        _CACHE[key] = _build_nc(IN_DT, OUT_DT)
    nc = _CACHE[key]

    import time
    from concourse.bass_utils import run_bass_kernel_spmd
    trace = bool(int(os.environ.get("KERNEL_TRACE", "0")))
    t0 = time.time()
    if trace:
        try:
            res = run_bass_kernel_spmd(nc, in_maps, core_ids=list(range(NCORES)),
                                       trace=True)
        except Exception:
            res = run_bass_kernel_spmd(nc, in_maps, core_ids=list(range(NCORES)))
    else:
        res = run_bass_kernel_spmd(nc, in_maps, core_ids=list(range(NCORES)))
    _CACHE["run_wall_s"] = time.time() - t0
    _CACHE["last_result"] = res

    # ---- reorder device rows into triu order, add overlap + bias
    mo_all = np.concatenate([res.results[m]["mo"] for m in range(NCORES)])
    dev_idx, inp_idx = _triu_maps()
    mo_inp = np.empty((P, BB), np.float32)
    mo_inp[inp_idx] = mo_all[dev_idx].astype(np.float32)
    return overlap.reshape(P, BB) + mo_inp + bo2
